# revision 12
# baseline (speedup 1.0000x reference)
"""GNN message-passing (SpMM + mean-normalize + bias) Trainium2 kernel.

out[r] = (sum_{e: rows[e]==r} vals[e] * x[cols[e]]) / deg[r] + bias,
deg[r] = sum vals[e], rows with deg==0 -> bias.

Strategy (8 NeuronCores, SPMD):
  - Pad N=40000 rows to 40960 = 320 bins x 128 rows. Core c owns bins
    [40c, 40c+40) => output rows [5120c, 5120(c+1)).  Edges are bucketed by
    destination bin on the host (this is the sharding step), so no
    cross-core collectives are needed.
  - Per bin, edges are split into a low group (col < 32768) and a high
    group (col >= 32768), each padded to a multiple of 128 with null
    edges (val=0), because dma_gather carries int16 indices.
  - Device per bin: two dma_gather ops fetch x rows for all edges
    (512B/row), slot i <- (partition i%128, chunk i//128).  For each
    128-edge chunk a one-hot selection matrix S[t,r] = (ri[t]==r)*val[t]
    is built on the vector engine from an iota tile, then the tensor
    engine computes psum[r,f] += S^T @ xg (PSUM accumulation) and
    deg[r] += S^T @ ones.  Epilogue normalizes by deg (deg==0 -> 0),
    adds bias, and DMAs the 128-row block out.
"""
import sys

sys.path.insert(0, "/opt/trn_rl_repo")

import numpy as np

N_NODES = 40000
N_EDGES = 640000
D = 128
P = 128
N_CORES = 8
BINS_PER_CORE = 40
N_BINS = N_CORES * BINS_PER_CORE          # 320 (rows padded to 40960)
SPLIT = 32768                             # int16-safe index split

_plan_cache: dict = {}


def _patch_ldw_opt():
    """Enable walrus's LDW dedup pass (second matmul on the same stationary
    S skips its LoadWeights)."""
    import concourse.bass_utils as bu

    if getattr(bu, "_ldw_patched", False):
        return
    orig = bu.run_command

    def patched(argv, **kw):
        argv = ["--enable-ldw-opt=true" if a == "--enable-ldw-opt=false" else a
                for a in argv]
        return orig(argv, **kw)

    bu.run_command = patched
    bu._ldw_patched = True


def _build_program(NLO, NHI, NXL, NXH):
    """Build+compile the SPMD Bass program for the given per-bin-position
    chunk schedule (shared by all cores)."""
    import concourse.bacc as bacc
    import concourse.bass as bass
    import concourse.tile as tile
    from concourse import mybir

    _patch_ldw_opt()

    NCH = [NLO[p] + NHI[p] for p in range(BINS_PER_CORE)]
    F = sum(NCH)
    F16 = F * 8

    NQ = 4
    nc = bacc.Bacc(num_swdge_queues=NQ)
    x_d = nc.dram_tensor("x", [N_NODES, D], mybir.dt.float32, kind="ExternalInput")
    idx_d = nc.dram_tensor("idx", [P, F16], mybir.dt.int16, kind="ExternalInput")
    meta_d = nc.dram_tensor("meta", [P, P + 4 * F + BINS_PER_CORE],
                            mybir.dt.float32, kind="ExternalInput")
    bias_d = nc.dram_tensor("bias", [P, D], mybir.dt.float32, kind="ExternalInput")
    degrow_d = nc.dram_tensor("degrow", [1, BINS_PER_CORE * P], mybir.dt.float32,
                              kind="ExternalInput")
    biasrow_d = nc.dram_tensor("biasrow", [1, D], mybir.dt.float32,
                               kind="ExternalInput")
    out_d = nc.dram_tensor("out", [BINS_PER_CORE * P, D], mybir.dt.float32,
                           kind="ExternalOutput")

    with tile.TileContext(nc) as tc:
        with tc.tile_pool(name="persist", bufs=1) as persist, \
             tc.tile_pool(name="xgp", bufs=5) as xgp, \
             tc.tile_pool(name="spool", bufs=12) as spool, \
             tc.tile_pool(name="outp", bufs=3) as outp, \
             tc.tile_pool(name="ep", bufs=2) as ep, \
             tc.tile_pool(name="actp", bufs=3) as actp, \
             tc.tile_pool(name="ps", bufs=4, space="PSUM") as ps, \
             tc.tile_pool(name="psd", bufs=2, space="PSUM") as psd:
            idx_t = persist.tile([P, F16], mybir.dt.int16)
            meta_t = persist.tile([P, P + 4 * F + BINS_PER_CORE],
                                  mybir.dt.float32)
            bias_t = persist.tile([P, D], mybir.dt.float32)
            degrow_t = persist.tile([1, BINS_PER_CORE * P], mybir.dt.float32)
            biasrow_t = persist.tile([1, D], mybir.dt.float32)
            ones_t = persist.tile([P, 1], mybir.dt.float32)
            nc.sync.dma_start(out=idx_t[:], in_=idx_d[:, :])
            nc.sync.dma_start(out=meta_t[:], in_=meta_d[:, :])
            nc.sync.dma_start(out=bias_t[:], in_=bias_d[:, :])
            nc.sync.dma_start(out=degrow_t[:], in_=degrow_d[:, :])
            nc.sync.dma_start(out=biasrow_t[:], in_=biasrow_d[:, :])
            nc.vector.memset(ones_t[:], 1.0)
            iota_t = meta_t[:, 0:P]

            maxch = max(NCH)
            for _w in range(5):
                wt = xgp.tile([P, maxch * D], mybir.dt.float32, tag="xg")
                nc.vector.memset(wt[:], 0.0)
            # dma_gather is limited to 1024 indices (8 chunks) per call
            GMAX = 8
            _gq = [0]
            for b in range(BINS_PER_CORE):
                offb = sum(NCH[:b])
                nch, nlo, nhi = NCH[b], NLO[b], NHI[b]
                xg = xgp.tile([P, nch * D], mybir.dt.float32, tag="xg")
                subs = []  # (chunk off, n chunks, is_high, exact idx count)
                for s in range(0, nlo, GMAX):
                    n = min(GMAX, nlo - s)
                    nidx = max(16, min(n * 128, NXL[b] - s * 128))
                    subs.append((s, n, False, nidx))
                for s in range(0, nhi, GMAX):
                    n = min(GMAX, nhi - s)
                    nidx = max(16, min(n * 128, NXH[b] - s * 128))
                    subs.append((nlo + s, n, True, nidx))
                for s, n, hi, nidx in subs:
                    nc.gpsimd.dma_gather(
                        out_ap=xg[:, s * D : (s + n) * D].rearrange(
                            "p (k w) -> p k w", k=n),
                        in_ap=(x_d[SPLIT:N_NODES, :] if hi else x_d[0:SPLIT, :]),
                        idxs_ap=idx_t[:, (offb + s) * 8 : (offb + s + n) * 8],
                        num_idxs=nidx,
                        num_idxs_reg=nidx,
                        elem_size=D,
                        queue_num=_gq[0] % NQ,
                    )
                    _gq[0] += 1
                psum = ps.tile([P, D], mybir.dt.float32, tag="psum")
                nc.tensor.matmul(out=psum[:],
                                 lhsT=degrow_t[:, b * P : (b + 1) * P],
                                 rhs=biasrow_t[:, :],
                                 start=True, stop=False)
                # tiny PE reads of xg: absorb the gather-DMA semaphore waits
                # so real matmuls carry only the DVE wait
                dummy = psd.tile([1, 1], mybir.dt.float32, tag="dummy")
                for s, n, hi, nidx in subs:
                    nc.tensor.matmul(out=dummy[:], lhsT=xg[:1, s * D : s * D + 1],
                                     rhs=xg[:1, s * D : s * D + 1],
                                     start=True, stop=True)
                NR0 = P + 2 * F + BINS_PER_CORE          # negri block offset
                NV0 = NR0 + F                             # negval block offset
                for c in range(nch):
                    S = spool.tile([P, P], mybir.dt.float32, tag="S")
                    if False:  # ACT S-build: correct but slower (act-table thrash)
                        # ACT path: S = Relu(val - val*(iota-ri)^2)
                        p1 = actp.tile([P, P], mybir.dt.float32, tag="p1")
                        nc.scalar.activation(
                            out=p1[:], in_=iota_t,
                            func=mybir.ActivationFunctionType.Square,
                            bias=meta_t[:, NR0 + offb + c : NR0 + offb + c + 1],
                            scale=1.0)
                        nc.scalar.activation(
                            out=S[:], in_=p1[:],
                            func=mybir.ActivationFunctionType.Relu,
                            bias=meta_t[:, P + F + offb + c : P + F + offb + c + 1],
                            scale=meta_t[:, NV0 + offb + c : NV0 + offb + c + 1])
                    elif c % 8 == 7:
                        nc.gpsimd.tensor_scalar(
                            out=S[:], in0=iota_t,
                            scalar1=meta_t[:, P + offb + c : P + offb + c + 1],
                            scalar2=meta_t[:, P + F + offb + c : P + F + offb + c + 1],
                            op0=mybir.AluOpType.is_equal, op1=mybir.AluOpType.mult,
                        )
                    else:
                        nc.vector.tensor_scalar(
                            out=S[:], in0=iota_t,
                            scalar1=meta_t[:, P + offb + c : P + offb + c + 1],
                            scalar2=meta_t[:, P + F + offb + c : P + F + offb + c + 1],
                            op0=mybir.AluOpType.is_equal, op1=mybir.AluOpType.mult,
                        )
                    nc.tensor.matmul(out=psum[:], lhsT=S[:],
                                     rhs=xg[:, c * D : (c + 1) * D],
                                     start=False, stop=(c == nch - 1))
                # epilogue: out = (agg + deg*bias) * rdeg  (on ACT)
                o_t = outp.tile([P, D], mybir.dt.float32, tag="o")
                nc.scalar.activation(
                    out=o_t[:], in_=psum[:],
                    func=mybir.ActivationFunctionType.Copy,
                    scale=meta_t[:, P + 2 * F + b : P + 2 * F + b + 1])
                nc.sync.dma_start(out=out_d[b * P : (b + 1) * P, :], in_=o_t[:])

    nc.compile()
    return nc


def _cdiv(a, b):
    return -(-a // b)


def _preprocess(x, edge_rows, edge_cols, adj_vals, bias):
    """Bucket edges by destination bin, split low/high cols, pad, and build
    per-core device input arrays."""
    bin_id = (edge_rows // P).astype(np.int64)
    is_high = (edge_cols >= SPLIT).astype(np.int64)
    order = np.lexsort((is_high, bin_id))
    b_s = bin_id[order]
    h_s = is_high[order]
    col_s = edge_cols[order].astype(np.int32)
    val_s = adj_vals[order].astype(np.float32)
    ri_s = (edge_rows[order] - b_s * P).astype(np.float32)

    n_tot = np.bincount(b_s, minlength=N_BINS)
    n_hi = np.bincount(b_s, weights=h_s, minlength=N_BINS).astype(np.int64)
    n_lo = n_tot - n_hi
    starts = np.concatenate([[0], np.cumsum(n_tot)])[:N_BINS]

    # per-position chunk counts, shared across cores (SPMD)
    NLO = [max(1, int(max(_cdiv(int(n_lo[40 * c + p]), P)
                          for c in range(N_CORES))))
           for p in range(BINS_PER_CORE)]
    NHI = [max(1, int(max(_cdiv(int(n_hi[40 * c + p]), P)
                          for c in range(N_CORES))))
           for p in range(BINS_PER_CORE)]
    NCH = [NLO[p] + NHI[p] for p in range(BINS_PER_CORE)]
    F = sum(NCH)
    NXL = [max(16, 16 * int(_cdiv(int(max(n_lo[40 * c + p] for c in range(N_CORES))), 16)))
           for p in range(BINS_PER_CORE)]
    NXH = [max(16, 16 * int(_cdiv(int(max(n_hi[40 * c + p] for c in range(N_CORES))), 16)))
           for p in range(BINS_PER_CORE)]

    iota_np = np.tile(np.arange(P, dtype=np.float32), (P, 1))
    bias_rep = np.tile(np.asarray(bias, dtype=np.float32), (P, 1))
    deg = np.bincount(edge_rows, weights=adj_vals.astype(np.float64),
                      minlength=N_BINS * P).astype(np.float32)
    rdeg = np.ones(N_BINS * P, np.float32)
    nz = deg != 0
    rdeg[nz] = (1.0 / deg[nz]).astype(np.float32)
    deg = deg.copy()
    deg[~nz] = 1.0

    in_maps = []
    for c in range(N_CORES):
        idx_parts = []
        ri_arr = np.zeros((P, F), np.float32)
        val_arr = np.zeros((P, F), np.float32)
        off = 0
        for p in range(BINS_PER_CORE):
            g = 40 * c + p
            s = int(starts[g])
            nl, nh = int(n_lo[g]), int(n_hi[g])
            lo_pad, hi_pad = NLO[p] * P, NHI[p] * P
            cols_lo = np.zeros(lo_pad, np.int32)
            cols_lo[:nl] = col_s[s : s + nl]
            cols_hi = np.full(hi_pad, SPLIT, np.int32)
            cols_hi[:nh] = col_s[s + nl : s + nl + nh]
            ris = np.zeros(lo_pad + hi_pad, np.float32)
            ris[:nl] = ri_s[s : s + nl]
            ris[lo_pad : lo_pad + nh] = ri_s[s + nl : s + nl + nh]
            vals = np.zeros(lo_pad + hi_pad, np.float32)
            vals[:nl] = val_s[s : s + nl]
            vals[lo_pad : lo_pad + nh] = val_s[s + nl : s + nl + nh]
            # wrapped int16 idx layout: idx i at [i%16, i//16], replicated 8x
            wlo = cols_lo.reshape(-1, 16).T.astype(np.int16)
            whi = (cols_hi - SPLIT).reshape(-1, 16).T.astype(np.int16)
            idx_parts.append(np.tile(wlo, (8, 1)))
            idx_parts.append(np.tile(whi, (8, 1)))
            nch = NCH[p]
            ri_arr[:, off : off + nch] = ris.reshape(nch, P).T
            val_arr[:, off : off + nch] = vals.reshape(nch, P).T
            off += nch
        idx_np = np.concatenate(idx_parts, axis=1)
        rdeg_arr = rdeg[5120 * c : 5120 * (c + 1)].reshape(BINS_PER_CORE, P).T
        meta_np = np.concatenate([iota_np, ri_arr, val_arr,
                                  np.ascontiguousarray(rdeg_arr),
                                  -ri_arr, -val_arr], axis=1)
        in_maps.append({
            "x": np.ascontiguousarray(x, dtype=np.float32),
            "idx": idx_np,
            "meta": meta_np,
            "bias": bias_rep,
            "degrow": np.ascontiguousarray(
                deg[5120 * c : 5120 * (c + 1)].reshape(1, -1)),
            "biasrow": np.asarray(bias, np.float32).reshape(1, -1),
        })
    return tuple(NLO), tuple(NHI), tuple(NXL), tuple(NXH), in_maps


def _run(x, edge_rows, edge_cols, adj_vals, bias, trace=False, trace_cores=None):
    from concourse.bass_utils import run_bass_kernel_spmd

    NLO, NHI, NXL, NXH, in_maps = _preprocess(x, edge_rows, edge_cols,
                                              adj_vals, bias)
    key = (NLO, NHI, NXL, NXH)
    if key not in _plan_cache:
        _plan_cache[key] = _build_program(list(NLO), list(NHI), list(NXL),
                                          list(NXH))
    nc = _plan_cache[key]
    kw = {}
    if trace:
        kw["trace"] = True
        if trace_cores is not None:
            kw["trace_cores"] = trace_cores
    res = run_bass_kernel_spmd(nc, in_maps, core_ids=list(range(N_CORES)), **kw)
    out = np.concatenate([res.results[c]["out"] for c in range(N_CORES)], axis=0)
    return out[:N_NODES].astype(np.float32), res


def kernel(x, edge_rows, edge_cols, adj_vals, bias):
    out, _ = _run(np.asarray(x), np.asarray(edge_rows), np.asarray(edge_cols),
                  np.asarray(adj_vals), np.asarray(bias))
    return out


# revision 13
# speedup vs baseline: 1.4231x; 1.4231x over previous
"""GNN message-passing (SpMM + mean-normalize + bias) Trainium2 kernel.

out[r] = (sum_{e: rows[e]==r} vals[e] * x[cols[e]]) / deg[r] + bias,
deg[r] = sum vals[e], rows with deg==0 -> bias.

Strategy (8 NeuronCores, SPMD):
  - Pad N=40000 rows to 40960 = 320 bins x 128 rows. Core c owns bins
    [40c, 40c+40) => output rows [5120c, 5120(c+1)).  Edges are bucketed by
    destination bin on the host (this is the sharding step), so no
    cross-core collectives are needed.
  - Per bin, edges are split into a low group (col < 32768) and a high
    group (col >= 32768), each padded to a multiple of 128 with null
    edges (val=0), because dma_gather carries int16 indices.
  - Device per bin: two dma_gather ops fetch x rows for all edges
    (512B/row), slot i <- (partition i%128, chunk i//128).  For each
    128-edge chunk a one-hot selection matrix S[t,r] = (ri[t]==r)*val[t]
    is built on the vector engine from an iota tile, then the tensor
    engine computes psum[r,f] += S^T @ xg (PSUM accumulation) and
    deg[r] += S^T @ ones.  Epilogue normalizes by deg (deg==0 -> 0),
    adds bias, and DMAs the 128-row block out.
"""
import sys

sys.path.insert(0, "/opt/trn_rl_repo")

import numpy as np

N_NODES = 40000
N_EDGES = 640000
D = 128
P = 128
N_CORES = 8
BINS_PER_CORE = 40
N_BINS = N_CORES * BINS_PER_CORE          # 320 (rows padded to 40960)
SPLIT = 32768                             # int16-safe index split

_plan_cache: dict = {}


def _patch_ldw_opt():
    """Enable walrus's LDW dedup pass (second matmul on the same stationary
    S skips its LoadWeights)."""
    import concourse.bass_utils as bu

    if getattr(bu, "_ldw_patched", False):
        return
    orig = bu.run_command

    def patched(argv, **kw):
        argv = ["--enable-ldw-opt=true" if a == "--enable-ldw-opt=false" else a
                for a in argv]
        return orig(argv, **kw)

    bu.run_command = patched
    bu._ldw_patched = True


def _build_program(NLO, NHI, NXL, NXH):
    """Build+compile the SPMD Bass program for the given per-bin-position
    chunk schedule (shared by all cores)."""
    import concourse.bacc as bacc
    import concourse.bass as bass
    import concourse.tile as tile
    from concourse import mybir

    _patch_ldw_opt()

    NCH = [NLO[p] + NHI[p] for p in range(BINS_PER_CORE)]
    F = sum(NCH)
    F16 = F * 8

    NQ = 4
    nc = bacc.Bacc(num_swdge_queues=NQ)
    x_d = nc.dram_tensor("x", [N_NODES, D], mybir.dt.float32, kind="ExternalInput")
    idx_d = nc.dram_tensor("idx", [P, F16], mybir.dt.int16, kind="ExternalInput")
    meta_d = nc.dram_tensor("meta", [P, P + 4 * F + BINS_PER_CORE],
                            mybir.dt.float32, kind="ExternalInput")
    bias_d = nc.dram_tensor("bias", [P, D], mybir.dt.float32, kind="ExternalInput")
    degrow_d = nc.dram_tensor("degrow", [1, BINS_PER_CORE * P], mybir.dt.float32,
                              kind="ExternalInput")
    biasrow_d = nc.dram_tensor("biasrow", [1, D], mybir.dt.float32,
                               kind="ExternalInput")
    out_d = nc.dram_tensor("out", [BINS_PER_CORE * P, D], mybir.dt.float32,
                           kind="ExternalOutput")

    with tile.TileContext(nc) as tc:
        with tc.tile_pool(name="persist", bufs=1) as persist, \
             tc.tile_pool(name="xgp", bufs=5) as xgp, \
             tc.tile_pool(name="spool", bufs=12) as spool, \
             tc.tile_pool(name="outp", bufs=3) as outp, \
             tc.tile_pool(name="ep", bufs=2) as ep, \
             tc.tile_pool(name="actp", bufs=3) as actp, \
             tc.tile_pool(name="ps", bufs=4, space="PSUM") as ps, \
             tc.tile_pool(name="psd", bufs=2, space="PSUM") as psd:
            idx_t = persist.tile([P, F16], mybir.dt.int16)
            meta_t = persist.tile([P, P + 4 * F + BINS_PER_CORE],
                                  mybir.dt.float32)
            bias_t = persist.tile([P, D], mybir.dt.float32)
            degrow_t = persist.tile([1, BINS_PER_CORE * P], mybir.dt.float32)
            biasrow_t = persist.tile([1, D], mybir.dt.float32)
            ones_t = persist.tile([P, 1], mybir.dt.float32)
            nc.sync.dma_start(out=idx_t[:], in_=idx_d[:, :])
            nc.sync.dma_start(out=meta_t[:], in_=meta_d[:, :])
            nc.sync.dma_start(out=bias_t[:], in_=bias_d[:, :])
            nc.sync.dma_start(out=degrow_t[:], in_=degrow_d[:, :])
            nc.sync.dma_start(out=biasrow_t[:], in_=biasrow_d[:, :])
            nc.vector.memset(ones_t[:], 1.0)
            iota_t = meta_t[:, 0:P]

            maxch = max(NCH)
            for _w in range(5):
                wt = xgp.tile([P, maxch * D], mybir.dt.float32, tag="xg")
                nc.vector.memset(wt[:], 0.0)
            # dma_gather is limited to 1024 indices (8 chunks) per call
            GMAX = 8
            _gq = [0]
            for b in range(BINS_PER_CORE):
                offb = sum(NCH[:b])
                nch, nlo, nhi = NCH[b], NLO[b], NHI[b]
                xg = xgp.tile([P, nch * D], mybir.dt.float32, tag="xg")
                subs = []  # (chunk off, n chunks, is_high, exact idx count)
                for s in range(0, nlo, GMAX):
                    n = min(GMAX, nlo - s)
                    nidx = max(16, min(n * 128, NXL[b] - s * 128))
                    subs.append((s, n, False, nidx))
                for s in range(0, nhi, GMAX):
                    n = min(GMAX, nhi - s)
                    nidx = max(16, min(n * 128, NXH[b] - s * 128))
                    subs.append((nlo + s, n, True, nidx))
                for s, n, hi, nidx in subs:
                    nc.gpsimd.dma_gather(
                        out_ap=xg[:, s * D : (s + n) * D].rearrange(
                            "p (k w) -> p k w", k=n),
                        in_ap=(x_d[SPLIT:N_NODES, :] if hi else x_d[0:SPLIT, :]),
                        idxs_ap=idx_t[:, (offb + s) * 8 : (offb + s + n) * 8],
                        num_idxs=nidx,
                        num_idxs_reg=nidx,
                        elem_size=D,
                        queue_num=_gq[0] % NQ,
                    )
                    _gq[0] += 1
                psum = ps.tile([P, D], mybir.dt.float32, tag="psum")
                nc.tensor.matmul(out=psum[:],
                                 lhsT=degrow_t[:, b * P : (b + 1) * P],
                                 rhs=biasrow_t[:, :],
                                 start=True, stop=False)
                # tiny PE reads of xg: absorb the gather-DMA semaphore waits
                # so real matmuls carry only the DVE wait
                dummy = psd.tile([1, 1], mybir.dt.float32, tag="dummy")
                for s, n, hi, nidx in subs:
                    nc.tensor.matmul(out=dummy[:], lhsT=xg[:1, s * D : s * D + 1],
                                     rhs=xg[:1, s * D : s * D + 1],
                                     start=True, stop=True)
                NR0 = P + 2 * F + BINS_PER_CORE          # negri block offset
                NV0 = NR0 + F                             # negval block offset
                for c in range(nch):
                    S = spool.tile([P, P], mybir.dt.float32, tag="S")
                    if False:  # ACT S-build: correct but slower (act-table thrash)
                        # ACT path: S = Relu(val - val*(iota-ri)^2)
                        p1 = actp.tile([P, P], mybir.dt.float32, tag="p1")
                        nc.scalar.activation(
                            out=p1[:], in_=iota_t,
                            func=mybir.ActivationFunctionType.Square,
                            bias=meta_t[:, NR0 + offb + c : NR0 + offb + c + 1],
                            scale=1.0)
                        nc.scalar.activation(
                            out=S[:], in_=p1[:],
                            func=mybir.ActivationFunctionType.Relu,
                            bias=meta_t[:, P + F + offb + c : P + F + offb + c + 1],
                            scale=meta_t[:, NV0 + offb + c : NV0 + offb + c + 1])
                    elif False:  # gpsimd S-build: ~2.4us/op, too slow
                        nc.gpsimd.tensor_scalar(
                            out=S[:], in0=iota_t,
                            scalar1=meta_t[:, P + offb + c : P + offb + c + 1],
                            scalar2=meta_t[:, P + F + offb + c : P + F + offb + c + 1],
                            op0=mybir.AluOpType.is_equal, op1=mybir.AluOpType.mult,
                        )
                    else:
                        nc.vector.tensor_scalar(
                            out=S[:], in0=iota_t,
                            scalar1=meta_t[:, P + offb + c : P + offb + c + 1],
                            scalar2=meta_t[:, P + F + offb + c : P + F + offb + c + 1],
                            op0=mybir.AluOpType.is_equal, op1=mybir.AluOpType.mult,
                        )
                    nc.tensor.matmul(out=psum[:], lhsT=S[:],
                                     rhs=xg[:, c * D : (c + 1) * D],
                                     start=False, stop=(c == nch - 1))
                # epilogue: out = (agg + deg*bias) * rdeg  (on ACT)
                o_t = outp.tile([P, D], mybir.dt.float32, tag="o")
                nc.scalar.activation(
                    out=o_t[:], in_=psum[:],
                    func=mybir.ActivationFunctionType.Copy,
                    scale=meta_t[:, P + 2 * F + b : P + 2 * F + b + 1])
                nc.sync.dma_start(out=out_d[b * P : (b + 1) * P, :], in_=o_t[:])

    nc.compile()
    return nc


def _cdiv(a, b):
    return -(-a // b)


def _preprocess(x, edge_rows, edge_cols, adj_vals, bias):
    """Bucket edges by destination bin, split low/high cols, pad, and build
    per-core device input arrays."""
    bin_id = (edge_rows // P).astype(np.int64)
    is_high = (edge_cols >= SPLIT).astype(np.int64)
    order = np.lexsort((is_high, bin_id))
    b_s = bin_id[order]
    h_s = is_high[order]
    col_s = edge_cols[order].astype(np.int32)
    val_s = adj_vals[order].astype(np.float32)
    ri_s = (edge_rows[order] - b_s * P).astype(np.float32)

    n_tot = np.bincount(b_s, minlength=N_BINS)
    n_hi = np.bincount(b_s, weights=h_s, minlength=N_BINS).astype(np.int64)
    n_lo = n_tot - n_hi
    starts = np.concatenate([[0], np.cumsum(n_tot)])[:N_BINS]

    # per-position chunk counts, shared across cores (SPMD)
    NLO = [max(1, int(max(_cdiv(int(n_lo[40 * c + p]), P)
                          for c in range(N_CORES))))
           for p in range(BINS_PER_CORE)]
    NHI = [max(1, int(max(_cdiv(int(n_hi[40 * c + p]), P)
                          for c in range(N_CORES))))
           for p in range(BINS_PER_CORE)]
    NCH = [NLO[p] + NHI[p] for p in range(BINS_PER_CORE)]
    F = sum(NCH)
    NXL = [max(16, 16 * int(_cdiv(int(max(n_lo[40 * c + p] for c in range(N_CORES))), 16)))
           for p in range(BINS_PER_CORE)]
    NXH = [max(16, 16 * int(_cdiv(int(max(n_hi[40 * c + p] for c in range(N_CORES))), 16)))
           for p in range(BINS_PER_CORE)]

    iota_np = np.tile(np.arange(P, dtype=np.float32), (P, 1))
    bias_rep = np.tile(np.asarray(bias, dtype=np.float32), (P, 1))
    deg = np.bincount(edge_rows, weights=adj_vals.astype(np.float64),
                      minlength=N_BINS * P).astype(np.float32)
    rdeg = np.ones(N_BINS * P, np.float32)
    nz = deg != 0
    rdeg[nz] = (1.0 / deg[nz]).astype(np.float32)
    deg = deg.copy()
    deg[~nz] = 1.0

    in_maps = []
    for c in range(N_CORES):
        idx_parts = []
        ri_arr = np.zeros((P, F), np.float32)
        val_arr = np.zeros((P, F), np.float32)
        off = 0
        for p in range(BINS_PER_CORE):
            g = 40 * c + p
            s = int(starts[g])
            nl, nh = int(n_lo[g]), int(n_hi[g])
            lo_pad, hi_pad = NLO[p] * P, NHI[p] * P
            cols_lo = np.zeros(lo_pad, np.int32)
            cols_lo[:nl] = col_s[s : s + nl]
            cols_hi = np.full(hi_pad, SPLIT, np.int32)
            cols_hi[:nh] = col_s[s + nl : s + nl + nh]
            ris = np.zeros(lo_pad + hi_pad, np.float32)
            ris[:nl] = ri_s[s : s + nl]
            ris[lo_pad : lo_pad + nh] = ri_s[s + nl : s + nl + nh]
            vals = np.zeros(lo_pad + hi_pad, np.float32)
            vals[:nl] = val_s[s : s + nl]
            vals[lo_pad : lo_pad + nh] = val_s[s + nl : s + nl + nh]
            # wrapped int16 idx layout: idx i at [i%16, i//16], replicated 8x
            wlo = cols_lo.reshape(-1, 16).T.astype(np.int16)
            whi = (cols_hi - SPLIT).reshape(-1, 16).T.astype(np.int16)
            idx_parts.append(np.tile(wlo, (8, 1)))
            idx_parts.append(np.tile(whi, (8, 1)))
            nch = NCH[p]
            ri_arr[:, off : off + nch] = ris.reshape(nch, P).T
            val_arr[:, off : off + nch] = vals.reshape(nch, P).T
            off += nch
        idx_np = np.concatenate(idx_parts, axis=1)
        rdeg_arr = rdeg[5120 * c : 5120 * (c + 1)].reshape(BINS_PER_CORE, P).T
        meta_np = np.concatenate([iota_np, ri_arr, val_arr,
                                  np.ascontiguousarray(rdeg_arr),
                                  -ri_arr, -val_arr], axis=1)
        in_maps.append({
            "x": np.ascontiguousarray(x, dtype=np.float32),
            "idx": idx_np,
            "meta": meta_np,
            "bias": bias_rep,
            "degrow": np.ascontiguousarray(
                deg[5120 * c : 5120 * (c + 1)].reshape(1, -1)),
            "biasrow": np.asarray(bias, np.float32).reshape(1, -1),
        })
    return tuple(NLO), tuple(NHI), tuple(NXL), tuple(NXH), in_maps


def _run(x, edge_rows, edge_cols, adj_vals, bias, trace=False, trace_cores=None):
    from concourse.bass_utils import run_bass_kernel_spmd

    NLO, NHI, NXL, NXH, in_maps = _preprocess(x, edge_rows, edge_cols,
                                              adj_vals, bias)
    key = (NLO, NHI, NXL, NXH)
    if key not in _plan_cache:
        _plan_cache[key] = _build_program(list(NLO), list(NHI), list(NXL),
                                          list(NXH))
    nc = _plan_cache[key]
    kw = {}
    if trace:
        kw["trace"] = True
        if trace_cores is not None:
            kw["trace_cores"] = trace_cores
    res = run_bass_kernel_spmd(nc, in_maps, core_ids=list(range(N_CORES)), **kw)
    out = np.concatenate([res.results[c]["out"] for c in range(N_CORES)], axis=0)
    return out[:N_NODES].astype(np.float32), res


def kernel(x, edge_rows, edge_cols, adj_vals, bias):
    out, _ = _run(np.asarray(x), np.asarray(edge_rows), np.asarray(edge_cols),
                  np.asarray(adj_vals), np.asarray(bias))
    return out
